# revision 18
# baseline (speedup 1.0000x reference)
"""Trainium2 Bass kernel for nn_Decoder_17214228922493.

32-step LSTM decoder: B=64, H=1536, input=1024, applied to a constant input.
    xg = x @ W_ih.T + b_ih + b_hh                      (once per step, see below)
    per step: gates = xg + h @ W_hh.T ; LSTM cell update ; emit h

Sharding: tensor-parallel over the gate dimension (8 cores x 768 gate
columns); after every step the 8 h^T slices are re-assembled with an
AllGather (mesh, ~6us).  Gate columns are reordered per core to
[f | o | i | g] so one sigmoid covers f,o (whose products with c can start
while the second matmul group is still streaming).

The xg contribution is re-computed from x every step instead of being
injected from a saved tile: those 18 matmuls have no dependency on h and
are explicitly held (add_dep_helper on the bounce DMA) so they execute
inside the AllGather window — free PE work off the critical path.

Matmul operands are bf16 (fp32 matmul costs 2 PE passes); PSUM
accumulation and the cell-state arithmetic stay fp32.
"""

import sys

if "/opt/trn_rl_repo" not in sys.path:
    sys.path.insert(0, "/opt/trn_rl_repo")

from contextlib import ExitStack

import ml_dtypes
import numpy as np

import concourse.bass as bass
import concourse.mybir as mybir
import concourse.tile as tile
from concourse import bacc
from concourse import bass_utils
from concourse._compat import get_trn_type

F32 = mybir.dt.float32
BF16 = mybir.dt.bfloat16
R = 8          # cores
B = 64         # batch
H = 1536       # hidden
HL = H // R    # 192 per-core hidden slice
IN = 1024      # lstm input size
KA = 1152      # augmented input contraction (1024 + bias row, padded to 9*128)
NG = 4 * HL    # 768 gate columns per core
S = 32         # steps
NH = 384       # matmul moving free-dim (two groups of 384 = NG)
KHT = H // 128   # 12 k-tiles for the recurrent matmul
KAT = KA // 128  # 9 k-tiles for the input matmul

_CACHE = {}


def _build():
    nc = bacc.Bacc(
        get_trn_type() or "TRN2",
        target_bir_lowering=False,
        debug=False,
        num_devices=R,
    )

    xT = nc.dram_tensor("xT", [KA, B], BF16, kind="ExternalInput")
    wih = nc.dram_tensor("wih", [KA, NG], BF16, kind="ExternalInput")
    whh = nc.dram_tensor("whh", [H, NG], BF16, kind="ExternalInput")
    h0T = nc.dram_tensor("h0T", [H, B], BF16, kind="ExternalInput")
    c0 = nc.dram_tensor("c0", [B, HL], F32, kind="ExternalInput")
    iden = nc.dram_tensor("iden", [B, B], BF16, kind="ExternalInput")
    out = nc.dram_tensor("out", [S, HL, B], BF16, kind="ExternalOutput")

    bounces = [
        nc.dram_tensor(f"bounce{t}", [HL, B], BF16, kind="Internal") for t in range(S)
    ]
    gaths = [
        nc.dram_tensor(f"gath{t}", [H, B], BF16, kind="Internal", addr_space="Shared")
        for t in range(S - 1)
    ]

    sig = mybir.ActivationFunctionType.Sigmoid
    tanh = mybir.ActivationFunctionType.Tanh

    with ExitStack() as ctx:
        tc = ctx.enter_context(tile.TileContext(nc))
        wpool = ctx.enter_context(tc.tile_pool(name="w", bufs=1))
        cpool = ctx.enter_context(tc.tile_pool(name="cst", bufs=1))
        hpool = ctx.enter_context(tc.tile_pool(name="h", bufs=2))
        spool = ctx.enter_context(tc.tile_pool(name="s", bufs=3))
        gpool = ctx.enter_context(tc.tile_pool(name="g", bufs=2, space="PSUM"))
        tpool = ctx.enter_context(tc.tile_pool(name="t", bufs=2, space="PSUM"))

        whh_t = []
        for k in range(KHT):
            w = wpool.tile([128, NG], BF16, tag=f"whh{k}")
            nc.sync.dma_start(w[:], whh[128 * k : 128 * (k + 1), :])
            whh_t.append(w)
        wih_t = []
        for k in range(KAT):
            w = wpool.tile([128, NG], BF16, tag=f"wih{k}")
            nc.sync.dma_start(w[:], wih[128 * k : 128 * (k + 1), :])
            wih_t.append(w)
        x_t = []
        for k in range(KAT):
            xx = wpool.tile([128, B], BF16, tag=f"x{k}")
            nc.sync.dma_start(xx[:], xT[128 * k : 128 * (k + 1), :])
            x_t.append(xx)
        iden_t = cpool.tile([B, B], BF16, tag="iden")
        nc.sync.dma_start(iden_t[:], iden[:])

        # h^T lives in three tiles [128, 4*B] (k-tiles 0-3 | 4-7 | 8-11) so the
        # post-AllGather reload is three chunked DMAs and the first matmuls can
        # start as soon as the first chunk lands.
        # reload chunk plan: a tiny first chunk (2 k-tiles) lands with the
        # smallest DMA+receipt latency so the first matmuls start early; the
        # rest arrives on a parallel ring while they run.
        chunk_plan = [(0, 2, 0), (2, 5, 1), (7, 5, 0)]  # (k0, nk, engine)
        reload_engines = [nc.sync, nc.scalar]

        def load_hT(src):
            chunks = {}
            for k0, nk, eng in chunk_plan:
                hc = hpool.tile([128, nk * B], BF16, tag=f"hh{k0}")
                src_ap = src.rearrange("(k p) n -> p k n", p=128)[
                    :, k0 : k0 + nk, :
                ]
                reload_engines[eng].dma_start(hc[:], src_ap)
                for k in range(k0, k0 + nk):
                    chunks[k] = (hc, k - k0)
            return chunks

        def h_tile(chunks, k):
            hc, off = chunks[k]
            return hc[:, B * off : B * (off + 1)]

        h_halves = load_hT(h0T)
        c_t = spool.tile([B, HL], F32, tag="c")
        nc.sync.dma_start(c_t[:], c0[:])

        prev_bounce_dma = None
        delay_gate = None  # dict of taps once the first AllGather exists
        for t in range(S):
            # gates = x^T.T@wih + h^T.T@whh in two 1-bank PSUM groups:
            # g0 = [f|o], g1 = [i|g].  The 9 x-matmuls per group have no h
            # dependency and fill the preceding AllGather window.
            ps = []
            for n in range(2):
                p = gpool.tile([B, NH], F32, tag=f"g{n}")
                nsl = bass.ts(n, NH)
                for k in range(KAT):
                    mm = nc.tensor.matmul(
                        p[:], x_t[k][:], wih_t[k][:, nsl], start=(k == 0), stop=False
                    )
                    if k == 0 and delay_gate is not None:
                        # pace the two x-matmul groups into the AllGather
                        # window (one at gather start, one at the tail) so no
                        # PE-idle stretch exceeds the ~3.4us HAM re-throttle
                        # threshold: the h-matmuls then enter at 2.4GHz
                        tile.add_dep_helper(
                            mm.ins,
                            delay_gate["early" if n == 0 else "late"].ins,
                            sync=True,
                            reason="x-matmuls paced into the AllGather window",
                        )
                ps.append(p)
            for n in range(2):
                nsl = bass.ts(n, NH)
                for k in range(KHT):
                    nc.tensor.matmul(
                        ps[n][:],
                        h_tile(h_halves, k),
                        whh_t[k][:, nsl],
                        start=False,
                        stop=(k == KHT - 1),
                    )

            # eltwise: group 0 = [g|f] finishes first -> tanh(g), sigmoid(f)
            # and f*c all run while group 1's h-matmuls still stream; group 1
            # = [o|i] needs a single wide sigmoid.
            tg = spool.tile([B, HL], F32, tag="tg")
            nc.scalar.activation(tg[:], ps[0][:, 0:HL], tanh)
            s_f = spool.tile([B, HL], F32, tag="sf")
            nc.scalar.activation(s_f[:], ps[0][:, HL : 2 * HL], sig)
            m1 = spool.tile([B, HL], F32, tag="m1")
            nc.vector.tensor_mul(m1[:], s_f[:], c_t[:])  # f*c
            s_o = spool.tile([B, HL], BF16, tag="so")
            nc.scalar.activation(s_o[:], ps[1][:, 0:HL], sig)
            s_i = spool.tile([B, HL], F32, tag="si")
            nc.scalar.activation(s_i[:], ps[1][:, HL : 2 * HL], sig)
            # transpose s_o early: off the critical c-chain (PE is free here)
            tp_so = tpool.tile([96, 2 * B], BF16, tag="hso")
            nc.tensor.transpose(tp_so[:, 0:B], s_o[:, 0:96], iden_t[:])
            nc.tensor.transpose(tp_so[:, B : 2 * B], s_o[:, 96:HL], iden_t[:])
            so_T = spool.tile([96, 2 * B], BF16, tag="soT")
            nc.vector.tensor_copy(so_T[:], tp_so[:])
            m2 = spool.tile([B, HL], F32, tag="m2")
            nc.vector.tensor_mul(m2[:], s_i[:], tg[:])  # i*g
            c_new = spool.tile([B, HL], F32, tag="c")
            nc.vector.tensor_add(c_new[:], m1[:], m2[:])
            c_t = c_new
            tc_sb = spool.tile([B, HL], BF16, tag="tc")
            nc.scalar.activation(tc_sb[:], c_new[:], tanh)
            tp_tc = tpool.tile([96, 2 * B], BF16, tag="htc_ps")
            nc.tensor.transpose(tp_tc[:, 0:B], tc_sb[:, 0:96], iden_t[:])
            nc.tensor.transpose(tp_tc[:, B : 2 * B], tc_sb[:, 96:HL], iden_t[:])

            # h^T = s_o^T * tanh(c)^T straight into the bounce-layout tile
            htc = spool.tile([96, 2 * B], BF16, tag="htc")
            nc.vector.tensor_mul(htc[:], so_T[:], tp_tc[:])
            dst = bounces[t].rearrange("(b p) n -> p b n", p=96)
            src = htc.rearrange("p (b n) -> p b n", b=2)
            prev_bounce_dma = nc.sync.dma_start(dst, src)

            if t < S - 1:
                nc.gpsimd.collective_compute(
                    "AllGather",
                    mybir.AluOpType.bypass,
                    replica_groups=[list(range(R))],
                    ins=[bounces[t][:]],
                    outs=[gaths[t][:]],
                )
                h_halves = load_hT(gaths[t])
                # paced DVE-only delay chain anchored on the bounce DMA with
                # two taps gating the next step's two x-matmul groups
                dprev = None
                delay_gate = {}
                for i in range(13):
                    dt_ = spool.tile([B, NG], BF16, tag="dly")
                    cp = nc.vector.tensor_copy(
                        dt_[:], wih_t[0][:B, :] if dprev is None else dprev[:]
                    )
                    if i == 0:
                        tile.add_dep_helper(
                            cp.ins,
                            prev_bounce_dma.ins,
                            sync=True,
                            reason="delay chain anchored to bounce",
                        )
                    if i == 3:
                        delay_gate["early"] = cp
                    if i == 12:
                        delay_gate["late"] = cp
                    dprev = dt_
            # final output written from the bounce copy, off the critical path
            # (after the collective trigger so it never delays it)
            nc.gpsimd.dma_start(out[t, :, :], bounces[t][:])

    nc.compile()
    return nc


def _prep_inputs(sequence, hidden_state, cell_state, W_ih, W_hh, b_ih, b_hh):
    x = np.asarray(sequence, np.float32)[0]          # [64, 1024]
    h0 = np.asarray(hidden_state, np.float32)[0]     # [64, 1536]
    c0f = np.asarray(cell_state, np.float32)[0]
    W_ih = np.asarray(W_ih, np.float32)
    W_hh = np.asarray(W_hh, np.float32)
    b = (np.asarray(b_ih, np.float32) + np.asarray(b_hh, np.float32))

    bf = ml_dtypes.bfloat16
    xT = np.zeros((KA, B), np.float32)
    xT[:IN] = x.T
    xT[IN] = 1.0
    xT = xT.astype(bf)
    h0T = np.ascontiguousarray(h0.T).astype(bf)
    iden = np.eye(B, dtype=bf)

    in_maps = []
    for r in range(R):
        sl = np.arange(r * HL, (r + 1) * HL)
        # per-core gate column order: g, f, o, i
        sel = np.concatenate([2 * H + sl, H + sl, 3 * H + sl, sl])
        wa = np.zeros((KA, NG), np.float32)
        wa[:IN] = W_ih[sel].T
        wa[IN] = b[sel]
        in_maps.append(
            {
                "xT": xT,
                "wih": wa.astype(bf),
                "whh": np.ascontiguousarray(W_hh[sel].T).astype(bf),
                "h0T": h0T,
                "c0": np.ascontiguousarray(c0f[:, sl]),
                "iden": iden,
            }
        )
    return in_maps


def kernel(**inputs) -> np.ndarray:
    if "nc" not in _CACHE:
        _CACHE["nc"] = _build()
    nc = _CACHE["nc"]
    in_maps = _prep_inputs(**inputs)
    res = bass_utils.run_bass_kernel_spmd(nc, in_maps, core_ids=list(range(R)))
    preds = np.empty((S, B, H), np.float32)
    for r in range(R):
        o = np.asarray(res.results[r]["out"], np.float32)  # [32, 192, 64]
        preds[:, :, r * HL : (r + 1) * HL] = np.transpose(o, (0, 2, 1))
    return preds
